# revision 20
# baseline (speedup 1.0000x reference)
"""Trainium2 Bass kernel for StyleGAN2-style upsampled Conv1d.

Reference computation (for x:(16,256,4096), weight:(256,256,3), bias:(256,)):
  y = conv_transpose1d(x, weight, stride=2)      # correlation on 2x-dilated x
  z = upfirdn1d(y, [1,3,3,1]/8 * 2)              # depthwise FIR
  out = z + bias                                  # (16, 256, 8192)

The transposed conv + FIR collapse into TWO 3-tap correlations over the
original x grid (even/odd output phases), with tap matrices
  even:  A  = .75w0+.25w1   B  = .25w0+.75w1+.75w2   C  = .25w2
  odd:   A' = .25w0         B' = .75w0+.75w1+.25w2   C' = .25w1+.75w2
Both rows sum to the same matrix WS := w0+w1+w2, so with the adjacent
diff e[i] = x[i] - x[i+1] and the shared stream V[j] = WS@x[j+1]:
  out_e = A @e[j-1] + (A +B )@e[j] + V + bias
  out_o = A'@e[j-1] + (A'+B')@e[j] + V + bias
which is 5 matmul streams (10 accumulating bf16 matmuls per 512-position
chunk: 2 K-tiles x {V->1 bank, 2 mats->even bank, 2 mats->odd bank})
instead of the direct form's 6 streams -- a 1/6 cut in PE work.  The e
diff is computed blockwise on gpsimd (SBUF-only engine, spare cycles)
right behind the x DMA stream; V drains through the scalar engine with
the bias folded in; even/odd banks drain to bf16 (scalar/vector), and
two 2x-mode vector adds produce the bf16 output chunk.  Output is
stored phase-separated [N,C,2,D]; the host interleaves (free numpy copy).
Chunk-major order keeps 2 PSUM pairs + 3 V banks rotating so drains
never stall the PE, and stores spread over the sync/scalar/gpsimd DMA
queues.  Accumulation stays fp32 in PSUM; bf16 rounding keeps rel err
~5e-3 (gate 2e-2).  Sharding: data-parallel over batch (2 per core x 8).
"""

import ml_dtypes
import numpy as np

import concourse.bass as bass
import concourse.mybir as mybir
import concourse.tile as tile
from concourse import bacc
from concourse.bass_utils import run_bass_kernel_spmd

N, IN_CH, OUT_CH, KERNEL, D = 16, 256, 256, 3, 4096
NCORES = 8
BPC = N // NCORES          # batches per core
DOUT = 2 * D
F32 = mybir.dt.float32
BF16 = mybir.dt.bfloat16
NPBF16 = ml_dtypes.bfloat16

NCHUNK = 512               # matmul moving free dim (= one PSUM bank of fp32)
NCHUNKS = D // NCHUNK      # 8
NMAT = 5                   # WS, A, B, A', B'

# x column blocks per SBUF tile; boundaries chosen even so the d1 diff
# ops keep 4-byte-aligned operands (2x DVE mode).  Fine-grained so the
# per-block d1/d2 prep pipelines tightly behind the x DMA stream.
XBLOCKS = [(0, NCHUNK + 4), (NCHUNK + 4, 2 * NCHUNK + 4),
           (2 * NCHUNK + 4, 4 * NCHUNK + 4), (4 * NCHUNK + 4, 6 * NCHUNK + 4),
           (6 * NCHUNK + 4, D + 2)]

_CACHED = {}


def _wblk(m, mat, k):
    # m-major so the first (m=0) half of the weights is one contiguous DMA
    return m * (2 * NMAT) + mat * 2 + k


def _build_nc(mm_dtype=BF16):
    nc = bacc.Bacc("TRN2", target_bir_lowering=False, debug=False)

    # x arrives host-padded with zero columns at 0 and D+1 (3-tap halo).
    x_t = nc.dram_tensor("x", [BPC, IN_CH, D + 2], BF16, kind="ExternalInput")
    w_t = nc.dram_tensor("w", [128, 2 * NMAT * 2 * 128], BF16, kind="ExternalInput")
    b_t = nc.dram_tensor("b", [128, 2], F32, kind="ExternalInput")
    # phase-separated output: [batch, ch, phase, pos]; host interleaves
    o_t = nc.dram_tensor("out", [BPC, OUT_CH, 2, D], BF16, kind="ExternalOutput")

    with tile.TileContext(nc) as tc:
        with (
            tc.tile_pool(name="wpool", bufs=1) as wpool,
            tc.tile_pool(name="xpool", bufs=2 * BPC) as xpool,
            tc.tile_pool(name="dpool", bufs=2 * BPC) as dpool,
            tc.tile_pool(name="zpool", bufs=6) as zpool,
            tc.tile_pool(name="vbpool", bufs=3) as vbpool,
            tc.tile_pool(name="ppool", bufs=2, space="PSUM") as ppool,
            tc.tile_pool(name="vpool", bufs=3, space="PSUM") as vpool,
        ):
            nw = 2 * NMAT * 2 * 128
            w_sb = wpool.tile([128, nw], mm_dtype)
            # m=0 weight half first: gates the very first matmul group
            nc.sync.dma_start(out=w_sb[:, :nw // 2], in_=w_t[:, :nw // 2])
            b_sb = wpool.tile([128, 2], F32)
            nc.sync.dma_start(out=b_sb[:], in_=b_t[:])
            nc.sync.dma_start(out=w_sb[:, nw // 2:], in_=w_t[:, nw // 2:])

            # x tiles (128, D+2) bf16, blockwise on the gpsimd SWDGE queue
            # (the only input path with real bandwidth; the HWDGE queues
            # measure 3-6x slower).
            x_sb = {}
            for bb in range(BPC):
                for k in range(2):
                    x_sb[bb, k] = xpool.tile(
                        [128, D + 2], mm_dtype, tag="x", name=f"x_{bb}_{k}"
                    )
            # adjacent-diff input e[i] = xp[i] - xp[i+1], computed blockwise
            # right behind each x block's DMA (gpsimd: SBUF-only engine
            # with spare cycles).  The DMA issue and the diff op for each
            # block are interleaved in program order so the gpsimd queue
            # streams x while earlier diffs compute.
            e_sb = {}
            for bb in range(BPC):
                for k in range(2):
                    e_sb[bb, k] = dpool.tile([128, D + 1], mm_dtype, tag="e",
                                             name=f"e_{bb}_{k}")
            for bb in range(BPC):
                for (lo, hi) in XBLOCKS:
                    dlo, dhi = (lo - 1, hi - 1) if lo else (0, hi - 1)
                    for k in range(2):
                        nc.gpsimd.dma_start(
                            out=x_sb[bb, k][:, lo:hi],
                            in_=x_t[bb, k * 128:(k + 1) * 128, lo:hi],
                        )
                    for k in range(2):
                        xs = x_sb[bb, k]
                        # bb0/k0 diffs ride the DVE (idle during startup)
                        # so the early chunks aren't rate-limited by the
                        # gpsimd queue, which also issues the x DMAs.
                        eng = nc.vector if (bb == 0 and k == 0) else nc.gpsimd
                        eng.tensor_sub(
                            e_sb[bb, k][:, dlo:dhi],
                            xs[:, dlo:dhi], xs[:, dlo + 1:dhi + 1],
                        )

            # Pre-warm the PE while inputs load: dummy bf16 matmuls on a
            # memset tile (no DMA dependency -- they start right after the
            # preamble) flip the HAM clock gate toward 8/8.  The PSUM
            # garbage lands in a pool slot that a later chunk's start=True
            # clears without reading.
            warm_bf = wpool.tile([128, 128 + NCHUNK], mybir.dt.bfloat16)
            nc.vector.memset(warm_bf[:], 1.0)
            warm_ps = vpool.tile([128, NCHUNK], F32, tag="v", name="warm_ps")
            for _ in range(12):
                nc.tensor.matmul(
                    warm_ps[:],
                    lhsT=warm_bf[:, 0:128],
                    rhs=warm_bf[:, 128:128 + NCHUNK],
                    start=True,
                    stop=True,
                )

            store_engines = [nc.scalar, nc.sync, nc.gpsimd, nc.gpsimd]
            chunk_no = 0
            zprev = None
            s0 = 0
            for bb in range(BPC):
                for m in range(2):
                    bias_ap = b_sb[:, m:m + 1]
                    for c in range(NCHUNKS):
                        s = c * NCHUNK
                        vps = vpool.tile([128, NCHUNK], F32, tag="v",
                                         name=f"v_{bb}_{m}_{c}")
                        pair = ppool.tile([128, 2 * NCHUNK], F32, tag="pair",
                                          name=f"pair_{bb}_{m}_{c}")
                        # V = WS @ x[j+1] first, so its drain overlaps the
                        # remaining 8 matmuls of the chunk.
                        for k in range(2):
                            nc.tensor.matmul(
                                vps[:],
                                lhsT=w_sb[:, _wblk(m, 0, k) * 128:][:, :128],
                                rhs=x_sb[bb, k][:, s + 2:s + 2 + NCHUNK],
                                start=(k == 0),
                                stop=(k == 1),
                            )
                        # even phase: A@e[j] + (A+B)@e[j+1]; odd phase:
                        # A'@e[j] + (A'+B')@e[j+1]
                        for phase in range(2):
                            for mat in range(2):      # 0 -> e[j], 1 -> e[j+1]
                                for k in range(2):
                                    nc.tensor.matmul(
                                        pair[:, phase * NCHUNK:(phase + 1) * NCHUNK],
                                        lhsT=w_sb[:, _wblk(m, 1 + 2 * phase + mat, k) * 128:][:, :128],
                                        rhs=e_sb[bb, k][:, s + mat:s + mat + NCHUNK],
                                        start=(mat == 0 and k == 0),
                                        stop=(mat == 1 and k == 1),
                                    )
                        # Epilogue: z_e = E + (V+bias), z_o = O + (V+bias).
                        # Measured rates: 1-input drains ~1.35 cyc/elem on
                        # either engine, 2-input TT ~1.4 (SBUF) / ~2.0
                        # (PSUM operand) cyc/elem and DVE-only.  Alternate
                        # two formulations per chunk so the DVE and ACT
                        # engines share the load evenly (~1.75us each per
                        # 2 chunks vs the 2.16us matmul chunk time).
                        vbf = vbpool.tile([128, NCHUNK], BF16, tag="vb",
                                          name=f"vb_{bb}_{m}_{c}")
                        nc.scalar.activation(
                            out=vbf[:], in_=vps[:],
                            func=mybir.ActivationFunctionType.Identity,
                            bias=bias_ap,
                        )
                        if chunk_no % 2 == 0:
                            zt = zpool.tile([128, 4 * NCHUNK], BF16, tag="z",
                                            name=f"z_{bb}_{m}_{c}")
                            zprev = zt
                            s0 = s
                            # direct combines from PSUM (DVE-heavy)
                            nc.vector.tensor_add(zt[:, :NCHUNK],
                                                 pair[:, :NCHUNK], vbf[:])
                            nc.vector.tensor_add(zt[:, 2 * NCHUNK:3 * NCHUNK],
                                                 pair[:, NCHUNK:], vbf[:])
                        else:
                            # drain-first combines (ACT-heavy)
                            ebf = vbpool.tile([128, NCHUNK], BF16, tag="eb",
                                              name=f"eb_{bb}_{m}_{c}")
                            nc.scalar.activation(
                                out=ebf[:], in_=pair[:, :NCHUNK],
                                func=mybir.ActivationFunctionType.Identity,
                            )
                            obf = vbpool.tile([128, NCHUNK], BF16, tag="ob",
                                              name=f"ob_{bb}_{m}_{c}")
                            nc.scalar.activation(
                                out=obf[:], in_=pair[:, NCHUNK:],
                                func=mybir.ActivationFunctionType.Identity,
                            )
                            zt = zprev
                            nc.vector.tensor_add(zt[:, NCHUNK:2 * NCHUNK],
                                                 ebf[:], vbf[:])
                            nc.vector.tensor_add(zt[:, 3 * NCHUNK:],
                                                 obf[:], vbf[:])
                            # two chunks per store: 2KB DRAM runs, spread
                            # over the gpsimd/sync/scalar queues
                            oeng = store_engines[(chunk_no // 2) % 4]
                            oeng.dma_start(
                                out=o_t[bb, m * 128:(m + 1) * 128, :,
                                        s0:s0 + 2 * NCHUNK],
                                in_=zt[:].rearrange("p (two j) -> p two j",
                                                    two=2),
                            )
                        chunk_no += 1
    nc.compile()
    return nc


def _host_weights(weight, bias):
    w = np.asarray(weight, dtype=np.float32)
    w0, w1, w2 = w[:, :, 0], w[:, :, 1], w[:, :, 2]
    A = 0.75 * w0 + 0.25 * w1
    B = 0.25 * w0 + 0.75 * w1 + 0.75 * w2
    Ap = 0.25 * w0
    Bp = 0.75 * w0 + 0.75 * w1 + 0.25 * w2
    mats = [w0 + w1 + w2, A, A + B, Ap, Ap + Bp]
    w_host = np.zeros((128, 2 * NMAT * 2 * 128), dtype=np.float32)
    for mat in range(NMAT):
        for k in range(2):
            for m in range(2):
                blk = _wblk(m, mat, k)
                # lhsT block[i, o] = M[m*128+o, k*128+i]
                wt = mats[mat][m * 128:(m + 1) * 128, k * 128:(k + 1) * 128]
                w_host[:, blk * 128:(blk + 1) * 128] = wt.T
    b_host = np.asarray(bias, dtype=np.float32).reshape(2, 128).T.copy()
    return w_host.astype(NPBF16), b_host


def _host_x(x):
    x = np.asarray(x, dtype=np.float32)
    return np.ascontiguousarray(
        np.pad(x, ((0, 0), (0, 0), (1, 1))).astype(NPBF16)
    )


def _host_out(res_list):
    # device layout is [BPC, C, 2, D] bf16 per core; interleave the two
    # phases into [N, C, 2D] fp32 on the host.
    dev = np.concatenate(
        [np.asarray(r["out"]).astype(np.float32) for r in res_list], axis=0
    )
    out = np.empty((N, OUT_CH, DOUT), dtype=np.float32)
    out[:, :, 0::2] = dev[:, :, 0, :]
    out[:, :, 1::2] = dev[:, :, 1, :]
    return out


def kernel(x, weight, bias):
    x = _host_x(x)
    w_host, b_host = _host_weights(weight, bias)

    if "nc" not in _CACHED:
        _CACHED["nc"] = _build_nc()
    nc = _CACHED["nc"]

    in_maps = []
    for core in range(NCORES):
        shard = np.ascontiguousarray(x[core * BPC:(core + 1) * BPC])
        in_maps.append({"x": shard, "w": w_host, "b": b_host})

    res = run_bass_kernel_spmd(nc, in_maps, core_ids=list(range(NCORES)))
    return _host_out(res.results)
